# Initial kernel scaffold
#
"""Causal multi-head attention (B=4, T=2048, H=16, hs=64, D=1024) on 8
Trainium2 NeuronCores.

Sharding: tensor-parallel over heads — each core computes 2 heads'
Q/K/V projections + attention, then a partial output projection
(y_partial = O_2h @ Wo[:, core_cols].T).  Host sums the 8 partials and
adds the bias (cheap: one fp32 reduction over 8 arrays).

On-core algorithm (per batch b, per head h):
  xT[b] [D,T] resident in SBUF (8 chunks of [128,T]).
  QT/KT/VT computed 2-head-packed: [128, T] = Wp.T @ xT  (PE, fp32r).
  V transposed per 128-key chunk via PE-transpose into Vtilde [128, 65]
  (65th column = ones, so the attention-times-V matmul also produces the
  softmax denominators).
  Scores are computed transposed, S_T [k=128, q=512] = KT_chunk.T @ QT_blk,
  exp'd pairwise on the scalar engine straight out of 2-bank PSUM tiles
  (scale=1/8 folded in; no max subtraction — scores are O(1)),
  causal-masked by one paired [128,1024] 0/1-mask multiply on GPSIMD
  (only diagonal chunk-pairs; strictly-upper chunks are skipped).
  O_T accumulates in PSUM: [65, 512] += Vtilde.T @ P_T over key chunks.
  Normalisation: reciprocal of the denominator row (DVE), GPSIMD
  partition_broadcast, multiply into OT_core.
  Output projection: y[b, 128-row chunk, :] = OT_core_chunk.T @ WoT_core,
  with PSUM evacuation split across DVE and ACT.
  Emission is software-pipelined: batch b+1's loads/projections are
  front-loaded into batch b's attention blocks so the PE stays fed.

All matmuls run as float32r (fp32 exponent, 11-bit mantissa) — full PE
rate at moving-dim 512.  Matmul operands are produced either by DMA from
pre-rounded host data or by compute-engine writes to f32r tiles.
"""

from contextlib import ExitStack

import numpy as np

import concourse.mybir as mybir
import concourse.tile as tile
from concourse import bacc

F32 = mybir.dt.float32
F32R = mybir.dt.float32r
EXP = mybir.ActivationFunctionType.Exp

# problem shape (hardcoded per harness contract)
B, T, D, H, HS = 4, 2048, 1024, 16, 64
N_CORES = 8
HPC = H // N_CORES          # heads per core = 2
QB = 512                    # query block (matmul moving dim)
KC = 128                    # key chunk (partition dim)
SCALE = HS ** -0.5


def round_fp32r(a: np.ndarray, mant_bits: int = 11) -> np.ndarray:
    """RNE-round fp32 to fp32r (11-bit mantissa kept, fp32 exponent)."""
    u = np.ascontiguousarray(a, dtype=np.float32).view(np.uint32)
    shift = np.uint32(23 - mant_bits)
    bias = ((u >> shift) & np.uint32(1)) + np.uint32((1 << (shift - 1)) - 1)
    u2 = ((u + bias) >> shift) << shift
    return u2.view(np.float32)


def build_nc(b=B, t=T, d=D, hpc=HPC, loop_n=1):
    """Build the per-core program. All cores run the same NEFF; per-core
    data (weight slices) comes in through the input tensors."""
    n_dc = d // 128           # D chunks (contraction for projections)
    n_qb = t // QB            # query blocks
    n_kc = t // KC            # key chunks
    n_tc = t // 128           # T chunks (output projection rows)
    mh = 64 * hpc             # packed head width (=128 for hpc=2)

    nc = bacc.Bacc("TRN2", target_bir_lowering=False, debug=False)

    xT = nc.dram_tensor("xT", [b, d, t], F32R, kind="ExternalInput").ap()
    wq = nc.dram_tensor("wq", [d, mh], F32R, kind="ExternalInput").ap()
    wk = nc.dram_tensor("wk", [d, mh], F32R, kind="ExternalInput").ap()
    wv = nc.dram_tensor("wv", [d, mh], F32R, kind="ExternalInput").ap()
    woT = nc.dram_tensor("woT", [mh, d], F32R, kind="ExternalInput").ap()
    masks = nc.dram_tensor("masks", [2, KC, 2 * QB], F32, kind="ExternalInput").ap()
    ident = nc.dram_tensor("ident", [128, 64], F32, kind="ExternalInput").ap()
    y = nc.dram_tensor("y", [b, t, d], F32, kind="ExternalOutput").ap()

    with tile.TileContext(nc) as tc, ExitStack() as ctx:
        consts = ctx.enter_context(tc.tile_pool(name="consts", bufs=1))
        xt_pool = ctx.enter_context(tc.tile_pool(name="xt", bufs=n_dc))
        qkv_pool = ctx.enter_context(tc.tile_pool(name="qkv", bufs=2))
        vtil_pool = ctx.enter_context(tc.tile_pool(name="vtil", bufs=2 * hpc))
        p_pool = ctx.enter_context(tc.tile_pool(name="p", bufs=3))
        ot_pool = ctx.enter_context(tc.tile_pool(name="ot", bufs=2))
        ysb_pool = ctx.enter_context(tc.tile_pool(name="ysb", bufs=2))
        small_pool = ctx.enter_context(tc.tile_pool(name="small", bufs=2))

        ps_proj = ctx.enter_context(tc.tile_pool(name="psp", bufs=2, space="PSUM"))
        ps_s = ctx.enter_context(tc.tile_pool(name="pss", bufs=2, space="PSUM"))
        ps_av = ctx.enter_context(tc.tile_pool(name="psav", bufs=2, space="PSUM"))

        # --- constants ---
        wq_sb = consts.tile([128, n_dc, mh], F32R, tag="wq")
        wk_sb = consts.tile([128, n_dc, mh], F32R, tag="wk")
        wv_sb = consts.tile([128, n_dc, mh], F32R, tag="wv")
        for w_sb, w_dram in ((wq_sb, wq), (wk_sb, wk), (wv_sb, wv)):
            nc.sync.dma_start(w_sb[:], w_dram.rearrange("(c p) m -> p c m", p=128))
        woT_sb = consts.tile([mh, d], F32R, tag="wo")
        nc.sync.dma_start(woT_sb[:], woT[:])
        masks_sb = consts.tile([KC, 2, 2 * QB], F32, tag="masks")
        nc.sync.dma_start(masks_sb[:], masks.rearrange("d p f -> p d f"))
        ident_sb = consts.tile([128, 64], F32, tag="ident")
        nc.sync.dma_start(ident_sb[:], ident[:])
        # ones column [128,1] for the Vtilde ones-column writes
        ones_f32 = consts.tile([128, 1], F32, tag="ones_f32")
        nc.vector.memset(ones_f32[:], 1.0)

        def make_proj_units(bi, st):
            """Load xT + QKV projections + Vtilde for batch bi, as a list
            of emission units (closures) to interleave with the previous
            batch's attention."""
            units = []

            def u_alloc():
                st["xt"] = []
                for c in range(n_dc):
                    xc = xt_pool.tile([128, t], F32R, tag="xt")
                    nc.sync.dma_start(xc[:], xT[bi, c * 128:(c + 1) * 128, :])
                    st["xt"].append(xc)
                st["qt2"] = qkv_pool.tile([mh, t], F32R, tag="qt2", name="qt2")
                st["kt2"] = qkv_pool.tile([mh, t], F32R, tag="kt2", name="kt2")
                st["vt2"] = qkv_pool.tile([mh, t], F32, tag="vt2", name="vt2")
            units.append(u_alloc)

            for key, wname in (("qt2", "wq"), ("kt2", "wk"), ("vt2", "wv")):
                for nb in range(n_qb):
                    def u_proj(key=key, wname=wname, nb=nb):
                        w_sb = {"wq": wq_sb, "wk": wk_sb, "wv": wv_sb}[wname]
                        dst = st[key]
                        acc = ps_proj.tile([mh, QB], F32, tag="proj")
                        for c in range(n_dc):
                            nc.tensor.matmul(
                                acc[:], w_sb[:, c, :],
                                st["xt"][c][:, nb * QB:(nb + 1) * QB],
                                start=(c == 0), stop=(c == n_dc - 1))
                        if key == "kt2":
                            nc.scalar.copy(dst[:, nb * QB:(nb + 1) * QB],
                                           acc[:])
                        else:
                            nc.vector.tensor_copy(
                                dst[:, nb * QB:(nb + 1) * QB], acc[:])
                    units.append(u_proj)

            for hh in range(hpc):
                def u_vtil(hh=hh):
                    vt = vtil_pool.tile([128, n_kc, 65], F32R, tag="vtil")
                    vt2 = st["vt2"]
                    # 8 transposes share one PSUM bank; a single strided
                    # copy evacuates all of them (fixed per-op DVE cost
                    # dominates, so fewer/bigger copies win)
                    for g in range(0, n_kc, 8):
                        gn = min(8, n_kc - g)
                        trp = ps_proj.tile([128, 512], F32, tag="proj")
                        for jj in range(gn):
                            j = g + jj
                            nc.tensor.transpose(
                                trp[:, jj * 64:(jj + 1) * 64],
                                vt2[hh * 64:(hh + 1) * 64,
                                    j * KC:(j + 1) * KC],
                                ident_sb[hh * 64:(hh + 1) * 64, :])
                        nc.vector.tensor_copy(
                            vt[:, g:g + gn, 0:64],
                            trp[:, 0:gn * 64].rearrange(
                                "p (j f) -> p j f", j=gn))
                    nc.vector.tensor_copy(
                        vt[:, :, 64], ones_f32[:].broadcast_to([128, n_kc]))
                    st[f"vtil{hh}"] = vt
                units.append(u_vtil)
            return units

        def make_attn_units(bi, st):
            """Attention + output projection for batch bi, one unit per
            query block."""
            units = []

            def u_attn(qb):
                if qb == 0:
                    st["ot"] = ot_pool.tile([mh, t], F32R, tag="ot", name="ot")
                ot_core = st["ot"]
                qt2, kt2 = st["qt2"], st["kt2"]
                kmax = (qb + 1) * (QB // KC)
                for hh in range(hpc):
                    qth = qt2[hh * 64:(hh + 1) * 64, :]
                    kth = kt2[hh * 64:(hh + 1) * 64, :]
                    vtil = st[f"vtil{hh}"]
                    oacc = ps_av.tile([128, QB], F32, tag="av")
                    for kc2 in range(kmax // 2):
                        # two score chunks share a 2-bank PSUM tile so one
                        # ACT instruction exps both (less per-op overhead)
                        sps = ps_s.tile([KC, 2 * QB], F32, tag="s")
                        for i in range(2):
                            kc = 2 * kc2 + i
                            nc.tensor.matmul(
                                sps[:, i * QB:(i + 1) * QB],
                                kth[:, kc * KC:(kc + 1) * KC],
                                qth[:, qb * QB:(qb + 1) * QB],
                                start=True, stop=True)
                        psb = p_pool.tile([KC, 2 * QB], F32R, tag="p")
                        nc.scalar.activation(psb[:], sps[:], EXP, scale=SCALE)
                        r = kc2 - 2 * qb
                        if r >= 0:
                            # diagonal pair: one mask multiply for both
                            # chunks; alternate engines to halve the
                            # exp->mask->AV chain's per-engine queueing
                            eng = nc.gpsimd if (hh + r) % 2 == 0 else nc.vector
                            eng.tensor_mul(psb[:], psb[:], masks_sb[:, r, :])
                        for i in range(2):
                            kc = 2 * kc2 + i
                            nc.tensor.matmul(
                                oacc[0:65, :], vtil[:, kc, :],
                                psb[:, i * QB:(i + 1) * QB],
                                start=(kc == 0), stop=(kc == kmax - 1))
                    # normalise: recip of denom row, partition-broadcast
                    # (gpsimd), multiply into ot_core
                    recf = small_pool.tile([1, QB], F32, tag="recf")
                    nc.vector.reciprocal(recf[:], oacc[64:65, :])
                    bcs = small_pool.tile([64, QB], F32, tag="bcs")
                    nc.gpsimd.partition_broadcast(bcs[:], recf[:])
                    nc.vector.tensor_mul(
                        ot_core[hh * 64:(hh + 1) * 64, qb * QB:(qb + 1) * QB],
                        oacc[0:64, :], bcs[:])

                # output projection for this query block's T-chunks
                for tcn in range(qb * (QB // 128), (qb + 1) * (QB // 128)):
                    ysb = ysb_pool.tile([128, d], F32, tag="ysb")
                    for nb0 in range(0, d, QB):
                        nw = min(QB, d - nb0)
                        op = ps_proj.tile([128, nw], F32, tag="proj")
                        nc.tensor.matmul(
                            op[:], ot_core[:, tcn * 128:(tcn + 1) * 128],
                            woT_sb[:, nb0:nb0 + nw],
                            start=True, stop=True)
                        if (tcn + nb0 // QB) % 2 == 0:
                            nc.vector.tensor_copy(ysb[:, nb0:nb0 + nw], op[:])
                        else:
                            nc.scalar.copy(ysb[:, nb0:nb0 + nw], op[:])
                    nc.sync.dma_start(y[bi, tcn * 128:(tcn + 1) * 128, :],
                                      ysb[:])

            for qb in range(n_qb):
                units.append(lambda qb=qb: u_attn(qb))
            return units

        def body():
            # software-pipelined emission: proj/load units of batch bi are
            # interleaved between the attention units of batch bi-1, so the
            # PE always has dense projection work to fill attention's
            # exp/mask dependency gaps.
            states = [dict() for _ in range(b)]
            prev_attn = None
            for bi in range(b):
                p_units = make_proj_units(bi, states[bi])
                a_units = make_attn_units(bi, states[bi])
                if prev_attn is None:
                    for u in p_units:
                        u()
                else:
                    # front-load next batch's proj into the early (cheap)
                    # attention blocks so proj(bi) is complete before the
                    # heavy last block of attn(bi-1); attn(bi) then overlaps
                    # that tail.
                    m = len(prev_attn)
                    k = len(p_units)
                    cuts = [0.4, 0.75, 1.0] + [1.0] * (m - 3)
                    emitted = 0
                    for j, au in enumerate(prev_attn):
                        au()
                        take = int(k * cuts[min(j, len(cuts) - 1)]) - emitted
                        for u in p_units[emitted:emitted + take]:
                            u()
                        emitted += take
                prev_attn = a_units
            for au in prev_attn:
                au()

        if loop_n > 1:
            with tc.For_i(0, loop_n, 1):
                body()
        else:
            body()

    nc.compile()
    return nc


_NC_CACHE = {}


def _get_nc():
    if "nc" not in _NC_CACHE:
        _NC_CACHE["nc"] = build_nc()
    return _NC_CACHE["nc"]


def make_masks() -> np.ndarray:
    """Two paired masks [KC, 2*QB]: pair 0 = [delta 0 | delta 128],
    pair 1 = [delta 256 | delta 384]."""
    m = np.zeros((2, KC, 2 * QB), np.float32)
    p = np.arange(KC)[:, None]
    f = np.arange(QB)[None, :]
    for pair in range(2):
        for half in range(2):
            dlt = (2 * pair + half) * KC
            m[pair][:, half * QB:(half + 1) * QB] = \
                (p + dlt <= f).astype(np.float32)
    return m


def make_in_maps(x, Wq, Wk, Wv, Wo):
    xTr = round_fp32r(np.ascontiguousarray(x.transpose(0, 2, 1)))
    masks = make_masks()
    ident = np.tile(np.eye(64, dtype=np.float32), (2, 1))
    in_maps = []
    for c in range(N_CORES):
        h0 = c * HPC
        wq2 = round_fp32r(Wq[h0:h0 + HPC].transpose(1, 0, 2).reshape(D, 64 * HPC))
        wk2 = round_fp32r(Wk[h0:h0 + HPC].transpose(1, 0, 2).reshape(D, 64 * HPC))
        wv2 = round_fp32r(Wv[h0:h0 + HPC].transpose(1, 0, 2).reshape(D, 64 * HPC))
        woT = round_fp32r(np.ascontiguousarray(
            Wo[:, h0 * 64:(h0 + HPC) * 64].T))
        in_maps.append({
            "xT": xTr, "wq": wq2, "wk": wk2, "wv": wv2, "woT": woT,
            "masks": masks, "ident": ident,
        })
    return in_maps


def kernel(x, Wq, Wk, Wv, Wo, bo):
    from concourse.bass_utils import run_bass_kernel_spmd

    x = np.asarray(x, np.float32)
    in_maps = make_in_maps(x, np.asarray(Wq, np.float32),
                           np.asarray(Wk, np.float32),
                           np.asarray(Wv, np.float32),
                           np.asarray(Wo, np.float32))
    nc = _get_nc()
    res = run_bass_kernel_spmd(nc, in_maps, core_ids=list(range(N_CORES)))
    out = res.results[0]["y"].astype(np.float64)
    for c in range(1, N_CORES):
        out += res.results[c]["y"]
    out += np.asarray(bo, np.float64)
    return out.astype(np.float32)



# revision 2
# speedup vs baseline: 1.1566x; 1.1566x over previous
"""Causal multi-head attention (B=4, T=2048, H=16, hs=64, D=1024) on 8
Trainium2 NeuronCores — v3 (all bf16).

Sharding: tensor-parallel over heads — each core computes 2 heads'
Q/K/V projections + attention + a partial output projection
(y_partial = O_2h @ Wo[:, core_cols].T, bf16).  Host sums the 8
partials in fp32 and adds the bias.

v2 changes vs v1 (807us baseline):
  * QKV projections run as fp8(e4m3) DoubleRow matmuls (2 rows/cycle,
    contraction 256/instruction): 82us -> 21us of PE time.  Weights are
    pre-scaled by 8 on the host so their mantissas sit in e4m3's normal
    range; the 1/64 compensation folds into the exp scale and Wo.
  * Output projection also DoubleRow fp8: both heads' attention outputs
    share PSUM partitions 0..63, so O is stored [64, 2(head), T] fp8 and
    contracts 128 dims in one 2-subtile instruction.
  * Everything else bf16 (Q/K/V tiles, P, Vtilde, masks, y partials):
    same PE rate as fp32r, half the SBUF/DMA traffic, DVE 2x mode on
    SBUF-only elementwise ops.
  * Attention inner loop software-pipelined with an explicit skew: the
    PE stream is S(0) S(1) AV(0) S(2) AV(1) ... so the exp of pair i
    finishes while the PE runs pair i+1's scores; PE never blocks on the
    in-order engine queue waiting for ACT.
  * Output-projection and next-batch projection units are woven between
    attention pairs so the PE has filler during normalize chains.
  * ACT runs exp only; all PSUM evacuations go to DVE/Pool, roughly
    load-balanced.
"""

from collections import deque
from contextlib import ExitStack

import numpy as np

import concourse.mybir as mybir
import concourse.tile as tile
from concourse import bacc

F32 = mybir.dt.float32
BF16 = mybir.dt.bfloat16
FP8 = mybir.dt.float8e4  # unused in v3
EXP = mybir.ActivationFunctionType.Exp
DR = mybir.MatmulPerfMode.DoubleRow

B, T, D, H, HS = 4, 2048, 1024, 16, 64
N_CORES = 8
HPC = H // N_CORES          # heads per core = 2
QB = 512                    # query block (matmul moving dim)
KC = 128                    # key chunk (partition dim)
EXP_SCALE = HS ** -0.5


def build_nc(b=B, t=T, d=D, hpc=HPC, loop_n=1):
    n_sub = d // 128          # 8 contraction subtiles
    n_qb = t // QB            # 4 query blocks
    n_kc = t // KC            # 16 key chunks
    mh = 64 * hpc             # packed head width (=128)

    nc = bacc.Bacc("TRN2", target_bir_lowering=False, debug=False)

    xdr = nc.dram_tensor("xdr", [b, n_sub, 128, t], BF16,
                         kind="ExternalInput").ap()
    wq = nc.dram_tensor("wq", [n_sub, 128, mh], BF16, kind="ExternalInput").ap()
    wk = nc.dram_tensor("wk", [n_sub, 128, mh], BF16, kind="ExternalInput").ap()
    wv = nc.dram_tensor("wv", [n_sub, 128, mh], BF16, kind="ExternalInput").ap()
    woT = nc.dram_tensor("woT", [mh, d], BF16, kind="ExternalInput").ap()
    masks = nc.dram_tensor("masks", [2, KC, 2 * QB], BF16,
                           kind="ExternalInput").ap()
    ident = nc.dram_tensor("ident", [128, 128], BF16, kind="ExternalInput").ap()
    y = nc.dram_tensor("y", [b, t, d], BF16, kind="ExternalOutput").ap()

    with tile.TileContext(nc) as tc, ExitStack() as ctx:
        consts = ctx.enter_context(tc.tile_pool(name="consts", bufs=1))
        xt_pool = ctx.enter_context(tc.tile_pool(name="xt", bufs=2))
        qkv_pool = ctx.enter_context(tc.tile_pool(name="qkv", bufs=2))
        vtil_pool = ctx.enter_context(tc.tile_pool(name="vtil", bufs=2 * hpc))
        p_pool = ctx.enter_context(tc.tile_pool(name="p", bufs=4))
        ot_pool = ctx.enter_context(tc.tile_pool(name="ot", bufs=2))
        ysb_pool = ctx.enter_context(tc.tile_pool(name="ysb", bufs=3))
        small_pool = ctx.enter_context(tc.tile_pool(name="small", bufs=4))

        ps_proj = ctx.enter_context(tc.tile_pool(name="psp", bufs=2,
                                                 space="PSUM"))
        ps_s = ctx.enter_context(tc.tile_pool(name="pss", bufs=2,
                                              space="PSUM"))
        ps_av = ctx.enter_context(tc.tile_pool(name="psav", bufs=2,
                                               space="PSUM"))

        # --- constants ---
        wq_sb = consts.tile([128, n_sub, mh], BF16, tag="wq")
        wk_sb = consts.tile([128, n_sub, mh], BF16, tag="wk")
        wv_sb = consts.tile([128, n_sub, mh], BF16, tag="wv")
        for w_sb, w_dram in ((wq_sb, wq), (wk_sb, wk), (wv_sb, wv)):
            nc.sync.dma_start(w_sb[:], w_dram.rearrange("s p m -> p s m"))
        woT_sb = consts.tile([mh, d], BF16, tag="wo")
        nc.sync.dma_start(woT_sb[:], woT[:])
        masks_sb = consts.tile([KC, 2, 2 * QB], BF16, tag="masks")
        nc.sync.dma_start(masks_sb[:], masks.rearrange("d p f -> p d f"))
        ident_sb = consts.tile([128, 128], BF16, tag="ident")
        nc.sync.dma_start(ident_sb[:], ident[:])
        ones_bf = consts.tile([128, 1], BF16, tag="ones_bf")
        nc.vector.memset(ones_bf[:], 1.0)

        def make_proj_units(bi, st):
            """Loads + QKV projections + Vtilde for batch bi as emission
            units, to interleave with the previous batch's attention."""
            units = []

            def u_load():
                xt = xt_pool.tile([128, n_sub, t], BF16, tag="xt")
                st["xt"] = xt
                for s in range(0, n_sub, 2):
                    nc.sync.dma_start(
                        xt[:, s:s + 2, :],
                        xdr[bi, s:s + 2, :, :].rearrange("s p t -> p s t"))
                st["qt2"] = qkv_pool.tile([mh, t], BF16, tag="qt2", name="qt2")
                st["kt2"] = qkv_pool.tile([mh, t], BF16, tag="kt2", name="kt2")
                st["vt2"] = qkv_pool.tile([mh, t], BF16, tag="vt2", name="vt2")
            units.append(u_load)

            for key, w_sb_name in (("qt2", "wq"), ("kt2", "wk"),
                                   ("vt2", "wv")):
                for nb in range(n_qb):
                    def u_proj(key=key, w_sb_name=w_sb_name, nb=nb):
                        w_sb = {"wq": wq_sb, "wk": wk_sb, "wv": wv_sb}[w_sb_name]
                        xt = st["xt"]
                        acc = ps_proj.tile([mh, QB], F32, tag="proj")
                        for s in range(n_sub):
                            nc.tensor.matmul(
                                acc[:], w_sb[:, s, :],
                                xt[:, s, nb * QB:(nb + 1) * QB],
                                start=(s == 0), stop=(s == n_sub - 1))
                        nc.vector.tensor_copy(
                            st[key][:, nb * QB:(nb + 1) * QB], acc[:])
                    units.append(u_proj)

            for g0 in (0, 4, 8, 12):
                def u_vtil(g0=g0):
                    if g0 == 0:
                        vt = vtil_pool.tile([128, n_kc, 132], BF16,
                                            tag="vtil", name="vt")
                        st["vtil"] = vt
                    vt = st["vtil"]
                    vt2 = st["vt2"]
                    trp = ps_proj.tile([mh, QB], BF16, tag="proj",
                                       name="trp")
                    for jj in range(4):
                        j = g0 + jj
                        # both heads at once: in [128, 128] -> out [128, 128]
                        nc.tensor.transpose(
                            trp[:, jj * 128:(jj + 1) * 128],
                            vt2[:, j * KC:(j + 1) * KC],
                            ident_sb[:])
                        # gap-split on evac: h0 -> cols 0..63, h1 -> 66..129
                        nc.vector.tensor_copy(
                            vt[:, j, 0:132].rearrange(
                                "p (g f) -> p g f", g=2, f=66)[:, :, 0:64],
                            trp[:, jj * 128:(jj + 1) * 128].rearrange(
                                "p (g f) -> p g f", g=2, f=64))
                    if g0 == 12:
                        nc.vector.tensor_copy(
                            vt[:, :, 64:132:66].rearrange("p k o -> p (k o)"),
                            ones_bf[:].broadcast_to([128, 2 * n_kc]))
                units.append(u_vtil)
            return units

        def make_attn_units(bi, st):
            """Attention + output projection for batch bi as a flat list
            of units with an explicit PE-pipeline skew."""
            SKEW = 2
            # (qb, hh, kc2) pair schedule
            pairs = [(qb, hh, kc2)
                     for qb in range(n_qb)
                     for hh in range(hpc)
                     for kc2 in range((qb + 1) * (QB // KC) // 2)]

            def emit_S(p):
                qb, hh, kc2 = p
                r = kc2 - 2 * qb
                # fully-masked column prefix of each chunk in a diagonal
                # pair (rectangle bound of the causal triangle)
                d0, d1 = (0, 0) if r < 0 else \
                    (2 * r * KC * (1 if r > 0 else 0), (2 * r + 1) * KC)
                qt2, kt2 = st["qt2"], st["kt2"]
                sps = ps_s.tile([KC, 2 * QB], F32, tag="s", name="sps")
                st[("sps", p)] = sps
                for i, dd in ((0, d0), (1, d1)):
                    kc = 2 * kc2 + i
                    nc.tensor.matmul(
                        sps[:, i * QB + dd:(i + 1) * QB],
                        kt2[hh * 64:(hh + 1) * 64, kc * KC:(kc + 1) * KC],
                        qt2[hh * 64:(hh + 1) * 64,
                            qb * QB + dd:(qb + 1) * QB],
                        start=True, stop=True)
                psb = p_pool.tile([KC, 2 * QB], BF16, tag="p", name="psb")
                st[("psb", p)] = psb
                if d0 == 0 and d1 == 0 and r < 0:
                    nc.scalar.activation(psb[:], sps[:], EXP, scale=EXP_SCALE)
                else:
                    # diagonal pair: exp + mask only the written regions
                    for i, dd in ((0, d0), (1, d1)):
                        lo, hi = i * QB + dd, (i + 1) * QB
                        nc.scalar.activation(psb[:, lo:hi], sps[:, lo:hi],
                                             EXP, scale=EXP_SCALE)
                        nc.gpsimd.tensor_mul(psb[:, lo:hi], psb[:, lo:hi],
                                             masks_sb[:, r, lo:hi])

            def emit_AV(p):
                qb, hh, kc2 = p
                kmax = (qb + 1) * (QB // KC)
                r = kc2 - 2 * qb
                if kc2 == 0:
                    st[("oacc", qb, hh)] = ps_av.tile(
                        [128, QB], F32, tag="av", name="oacc")
                oacc = st[("oacc", qb, hh)]
                vtil = st["vtil"]
                psb = st.pop(("psb", p))
                st.pop(("sps", p))
                for i in range(2):
                    kc = 2 * kc2 + i
                    dd = 0 if r < 0 else (2 * r + i) * KC
                    if kc == 0:
                        dd = 0  # start chunk must clear the whole bank
                    nc.tensor.matmul(
                        oacc[0:65, dd:],
                        vtil[:, kc, 66 * hh:66 * hh + 65],
                        psb[:, i * QB + dd:(i + 1) * QB],
                        start=(kc == 0), stop=(kc == kmax - 1))

            def emit_norm(qb, hh):
                if hh == 0 and qb == 0:
                    st["ot"] = ot_pool.tile([mh, t], BF16, tag="ot", name="ot")
                oacc = st.pop(("oacc", qb, hh))
                recf = small_pool.tile([1, QB], F32, tag="recf")
                nc.vector.reciprocal(recf[:], oacc[64:65, :])
                bcs = small_pool.tile([64, QB], F32, tag="bcs")
                nc.gpsimd.partition_broadcast(bcs[:], recf[:])
                nc.vector.tensor_mul(
                    st["ot"][hh * 64:(hh + 1) * 64, qb * QB:(qb + 1) * QB],
                    oacc[0:64, :], bcs[:])

            evac_flip = [0]

            def outproj_units(qb):
                """Per T-chunk: 2 DoubleRow matmuls + 2 evacs + y DMA,
                split into two units."""
                units = []
                for tcn in range(qb * (QB // 128), (qb + 1) * (QB // 128)):
                    def u_mm(tcn=tcn):
                        ot = st["ot"]
                        ysb = ysb_pool.tile([128, d], BF16, tag="ysb",
                                             name="ysb")
                        st[("ysb", tcn)] = ysb
                        for half in range(2):
                            op = ps_proj.tile([128, QB], F32, tag="proj")
                            nc.tensor.matmul(
                                op[:], ot[:, tcn * 128:(tcn + 1) * 128],
                                woT_sb[:, half * QB:(half + 1) * QB],
                                start=True, stop=True)
                            nc.vector.tensor_copy(
                                ysb[:, half * QB:(half + 1) * QB], op[:])

                    def u_dma(tcn=tcn):
                        ysb = st.pop(("ysb", tcn))
                        nc.sync.dma_start(
                            y[bi, tcn * 128:(tcn + 1) * 128, :], ysb[:])
                    units.append(u_mm)
                    units.append(u_dma)
                return units

            units = []
            filler = deque()

            def drain_one_filler():
                if filler:
                    filler.popleft()()

            for idx, p in enumerate(pairs):
                def u_pair(idx=idx, p=p):
                    emit_S(p)
                    if idx >= SKEW:
                        pprev = pairs[idx - SKEW]
                        emit_AV(pprev)
                        qbp, hhp, kc2p = pprev
                        if kc2p == (qbp + 1) * (QB // KC) // 2 - 1:
                            emit_norm(qbp, hhp)
                            if hhp == hpc - 1:
                                filler.extend(outproj_units(qbp))
                    drain_one_filler()
                units.append(u_pair)

            def u_tail():
                for p in pairs[-SKEW:]:
                    emit_AV(p)
                    qbp, hhp, kc2p = p
                    if kc2p == (qbp + 1) * (QB // KC) // 2 - 1:
                        emit_norm(qbp, hhp)
                        if hhp == hpc - 1:
                            filler.extend(outproj_units(qbp))
                while filler:
                    drain_one_filler()
            units.append(u_tail)
            return units

        def body():
            states = [dict() for _ in range(b)]
            prev_attn = None
            for bi in range(b):
                p_units = make_proj_units(bi, states[bi])
                a_units = make_attn_units(bi, states[bi])
                if prev_attn is None:
                    for u in p_units:
                        u()
                else:
                    # weave proj(bi) into attn(bi-1): loads first, then
                    # one proj unit after every other attention pair.
                    m = len(prev_attn)
                    k = len(p_units)
                    emitted = 0
                    for j, au in enumerate(prev_attn):
                        au()
                        want = min(k, 1 + ((j + 1) * (k - 1)) // max(1, m - 8))
                        while emitted < want:
                            p_units[emitted]()
                            emitted += 1
                    while emitted < k:
                        p_units[emitted]()
                        emitted += 1
                prev_attn = a_units
            for au in prev_attn:
                au()

        if loop_n > 1:
            with tc.For_i(0, loop_n, 1):
                body()
        else:
            body()

    nc.compile()
    return nc


_NC_CACHE = {}


def _get_nc():
    if "nc" not in _NC_CACHE:
        _NC_CACHE["nc"] = build_nc()
    return _NC_CACHE["nc"]


def make_masks() -> np.ndarray:
    """Two paired masks [KC, 2*QB]: pair 0 = [delta 0 | delta 128],
    pair 1 = [delta 256 | delta 384]."""
    m = np.zeros((2, KC, 2 * QB), np.float32)
    p = np.arange(KC)[:, None]
    f = np.arange(QB)[None, :]
    for pair in range(2):
        for half in range(2):
            dlt = (2 * pair + half) * KC
            m[pair][:, half * QB:(half + 1) * QB] = \
                (p + dlt <= f).astype(np.float32)
    return m


def to_fp8(a: np.ndarray) -> np.ndarray:
    import ml_dtypes
    return np.clip(np.asarray(a, np.float32), -240.0, 240.0).astype(
        ml_dtypes.float8_e4m3)


def to_bf16(a: np.ndarray) -> np.ndarray:
    import ml_dtypes
    return np.asarray(a, np.float32).astype(ml_dtypes.bfloat16)


def make_in_maps(x, Wq, Wk, Wv, Wo):
    x = np.asarray(x, np.float32)
    n_sub = D // 128
    # xdr[b, s, k, t] = x[b, t, 128 s + k]
    xdr = to_bf16(np.ascontiguousarray(
        x.transpose(0, 2, 1).reshape(B, n_sub, 128, T)))
    masks = to_bf16(make_masks())
    ident = to_bf16(np.eye(128, dtype=np.float32))
    in_maps = []
    for c in range(N_CORES):
        h0 = c * HPC
        def wdr(W):
            w2 = W[h0:h0 + HPC].transpose(1, 0, 2).reshape(D, 64 * HPC)
            return to_bf16(w2.reshape(n_sub, 128, 64 * HPC))
        woT_c = to_bf16(np.ascontiguousarray(
            Wo[:, h0 * 64:(h0 + HPC) * 64].T))
        in_maps.append({
            "xdr": xdr, "wq": wdr(Wq), "wk": wdr(Wk), "wv": wdr(Wv),
            "woT": woT_c, "masks": masks, "ident": ident,
        })
    return in_maps


def kernel(x, Wq, Wk, Wv, Wo, bo):
    from concourse.bass_utils import run_bass_kernel_spmd

    in_maps = make_in_maps(x, np.asarray(Wq, np.float32),
                           np.asarray(Wk, np.float32),
                           np.asarray(Wv, np.float32),
                           np.asarray(Wo, np.float32))
    nc = _get_nc()
    res = run_bass_kernel_spmd(nc, in_maps, core_ids=list(range(N_CORES)))
    out = res.results[0]["y"].astype(np.float64)
    for c in range(1, N_CORES):
        out += res.results[c]["y"].astype(np.float64)
    out += np.asarray(bo, np.float64)
    return out.astype(np.float32)
